# revision 15
# baseline (speedup 1.0000x reference)
"""GAT layer (2-relation HeteroGraphConv) Trainium2 kernel.

Sharding: 8 cores = 2 relations x 4 dst-quarters. Each core aggregates all
edges of one relation whose dst falls in its quarter of nodes; host sums the
two relu'd relation outputs per quarter.

Device pipeline per core:
  P1: build zal table = feat_src @ (W * al) in fp16 (head-major permuted cols)
      + er vector per dst node (feat_dst @ (W * ar), head-summed).
  P2: per 128-edge tile: dma_gather zal rows by src; el = grouped reduce;
      er per edge via one-hot matmul; ex = exp(prelu(el+er)); one-hot
      aggregation matmul accumulates [numer | denom] per dst block in PSUM,
      flushed into an SBUF accumulator.
  P3: out = relu(numer / denom / al + feat_dst + bias), DMA to HBM.
"""
import sys, os
sys.path.insert(0, "/opt/trn_rl_repo")
PH = os.environ.get("KPH", "123")
SUB = os.environ.get("KSUB", "gbex")
import numpy as np
import concourse.bass as bass
import concourse.mybir as mybir
from concourse import bacc
from concourse.tile import TileContext
from concourse.bass_utils import run_bass_kernel_spmd
from concourse import library_config

F32 = mybir.dt.float32
F16 = mybir.dt.float16
I16 = mybir.dt.int16
AF = mybir.ActivationFunctionType
ALU = mybir.AluOpType

# ---------------- problem config (full size; test_small overrides) ---------
CFG = dict(
    N=100000,      # nodes per type
    E=1600000,     # edges per relation
    IN=128, H=4, D=32,
    SLOPE=0.2,
)
ST = 16            # tiles per stage (gather granularity = 128*ST edges)


def _derived(cfg):
    N = cfg["N"]
    NQ = N // 4                      # real nodes per quarter
    NB = (NQ + 127) // 128           # dst blocks per quarter
    NQP = NB * 128                   # padded quarter
    WS = ((N + 3) // 4 + 127) // 128 * 128   # window size (padded)
    NT = 4 * WS                      # padded table rows
    assert WS <= 32767
    return NQ, NB, NQP, WS, NT


def host_prep(inputs, cfg=CFG):
    """Returns (in_maps, meta). meta holds the shared program structure."""
    N, E, IN, H, D = cfg["N"], cfg["E"], cfg["IN"], cfg["H"], cfg["D"]
    NQ, NB, NQP, WS, NT = _derived(cfg)
    NW = 4

    f_item = np.asarray(inputs["feat_item"], np.float32)
    f_tgt = np.asarray(inputs["feat_target"], np.float32)

    rel = [dict(src=np.asarray(inputs["src_i2t"]), dst=np.asarray(inputs["dst_i2t"]),
                W=np.asarray(inputs["W_i2t"], np.float32), al=np.asarray(inputs["al_i2t"], np.float32),
                ar=np.asarray(inputs["ar_i2t"], np.float32), b=np.asarray(inputs["b_i2t"], np.float32),
                feat_src=f_item),
           dict(src=np.asarray(inputs["src_t2t"]), dst=np.asarray(inputs["dst_t2t"]),
                W=np.asarray(inputs["W_t2t"], np.float32), al=np.asarray(inputs["al_t2t"], np.float32),
                ar=np.asarray(inputs["ar_t2t"], np.float32), b=np.asarray(inputs["b_t2t"], np.float32),
                feat_src=f_tgt)]

    # head-major column permutation: pcol[d*4+h] = h*32+d
    perm = (np.arange(D)[:, None] * 0 + np.arange(H)[None, :] * D
            + np.arange(D)[:, None]).reshape(-1)  # [D*H] -> value h*D+d at (d*H+h)

    # --- per-core edge structuring ------------------------------------
    cores = []
    for r in range(2):
        src, dst = rel[r]["src"].astype(np.int64), rel[r]["dst"].astype(np.int64)
        for q in range(4):
            qlo, qhi = NQ * q, NQ * (q + 1)
            m = (dst >= qlo) & (dst < qhi)
            s, d = src[m], dst[m] - qlo
            w = s // WS
            b = d // 128
            order = np.lexsort((s, d, w))
            s, d, w, b = s[order], d[order], w[order], b[order]
            cores.append(dict(s=s, d=d, w=w, b=b, r=r, q=q))

    # slot tile counts shared across cores: T[w][b]
    T = np.zeros((NW, NB), np.int64)
    for co in cores:
        cnt = np.zeros((NW, NB), np.int64)
        np.add.at(cnt, (co["w"], co["b"]), 1)
        T = np.maximum(T, (cnt + 127) // 128)

    # tile plan (shared): per window: list of (b, n_tiles); stages of ST tiles
    meta = dict(T=T, NW=NW, NB=NB, NQP=NQP, WS=WS, NT=NT, cfg=cfg)
    win_tiles = [int(T[w].sum()) for w in range(NW)]
    meta["win_tiles"] = win_tiles
    # tile -> slot block, slot-first, slot-last flags, per window
    tile_block, tile_first, tile_last = [], [], []
    for w in range(NW):
        tb, tf, tl = [], [], []
        for b in range(NB):
            t = int(T[w][b])
            for i in range(t):
                tb.append(b); tf.append(i == 0); tl.append(i == t - 1)
        tile_block.append(tb); tile_first.append(tf); tile_last.append(tl)
    meta["tile_block"] = tile_block
    meta["tile_first"] = tile_first
    meta["tile_last"] = tile_last
    # first window with edges per block (for copy-vs-add flush); -1 if none
    firstw = [-1] * NB
    for b in range(NB):
        for w in range(NW):
            if T[w][b] > 0:
                firstw[b] = w
                break
    meta["firstw"] = firstw
    ntile_tot = sum(win_tiles)
    EP = 128 * ntile_tot
    meta["EP"] = EP
    nstg = [int((wt + ST - 1) // ST) for wt in win_tiles]
    meta["nstg"] = nstg
    NSTG = sum(nstg)
    meta["NSTG"] = NSTG

    # --- per-core arrays ----------------------------------------------
    in_maps = []
    for co in cores:
        r, q = co["r"], co["q"]
        R = rel[r]
        # padded edge arrays in final order
        s_rel = np.zeros(EP, np.int16)
        d_rel = np.full(EP, -1.0, np.float32)
        pos = 0
        # group core's edges by (w, b) in sorted order; emit with slot padding
        s_, d_, w_, b_ = co["s"], co["d"], co["w"], co["b"]
        # index ranges per (w,b) via searchsorted on (w*NB + b) keys
        keys = w_ * NB + b_
        for w in range(NW):
            for b in range(NB):
                t = int(T[w][b])
                if t == 0:
                    continue
                k = w * NB + b
                lo = np.searchsorted(keys, k, "left")
                hi = np.searchsorted(keys, k, "right")
                n = hi - lo
                s_rel[pos:pos + n] = (s_[lo:hi] - w * WS).astype(np.int16)
                d_rel[pos:pos + n] = (d_[lo:hi] - b * 128).astype(np.float32)
                pos += 128 * t
        assert pos == EP
        # pack gather idx per stage: [128, EP/16]
        idxp = np.zeros((128, EP // 16), np.int16)
        dcol = np.zeros((NSTG, 128, ST), np.float32)
        drow = np.zeros((NSTG, 128 * ST), np.float16)
        a = 0       # edge offset
        sg = 0      # global stage idx
        for w in range(NW):
            wt = win_tiles[w]
            for s0 in range(0, wt, ST):
                n_t = min(ST, wt - s0)
                n_e = 128 * n_t
                seg = s_rel[a:a + n_e]
                idxp[:, a // 16:(a + n_e) // 16] = np.tile(
                    seg.reshape(-1, 16).T, (8, 1))
                dseg = d_rel[a:a + n_e]
                dcol[sg, :, :n_t] = dseg.reshape(n_t, 128).T
                drow[sg, :n_e] = dseg.astype(np.float16)
                a += n_e
                sg += 1
        assert a == EP and sg == NSTG

        W, al, ar, b_p = R["W"], R["al"], R["ar"], R["b"]
        al_f = al.reshape(-1)            # [H*D] order h*D+d
        ar_f = ar.reshape(-1)
        Wz = (W * al_f[None, :])[:, perm]          # permuted cols (d*H+h)
        War = W * ar_f[None, :]                    # plain cols
        inval = (1.0 / al_f).astype(np.float32)    # plain order

        fsrc16 = R["feat_src"].astype(np.float16)
        tblT = np.zeros((128, NT), np.float16)
        tblT[:, :N] = fsrc16.T
        ftgt16 = f_tgt.astype(np.float16)
        fq16T = np.zeros((128, NQP), np.float16)
        fq16T[:, :NQ] = ftgt16[NQ * q:NQ * (q + 1)].T
        fq32 = np.zeros((128, NB * 128), np.float32)
        fq = f_tgt[NQ * q:NQ * (q + 1)]            # [NQ, 128]
        fq_pad = np.zeros((NQP, IN), np.float32)
        fq_pad[:NQ] = fq
        fq32[:, :] = fq_pad.reshape(NB, 128, IN).transpose(1, 0, 2).reshape(128, -1)

        in_maps.append(dict(
            tblT=tblT,
            fq16T=fq16T,
            fq32=fq32,
            Wz16=Wz.astype(np.float16),
            War16=War.astype(np.float16),
            bias_b=np.broadcast_to(b_p.astype(np.float32), (128, IN)).copy(),
            inval_b=np.broadcast_to(inval, (128, IN)).copy(),
            iota_b16=np.broadcast_to(np.arange(128, dtype=np.float16), (128, 128)).copy(),
            iota_colf=np.arange(128, dtype=np.float32).reshape(128, 1),
            idxp=idxp,
            dcol=dcol,
            drow=drow,
        ))
    return in_maps, meta


def build_program(meta):
    cfg = meta["cfg"]
    IN, H, D = cfg["IN"], cfg["H"], cfg["D"]
    NW, NB, NQP, WS, NT = meta["NW"], meta["NB"], meta["NQP"], meta["WS"], meta["NT"]
    EP, NSTG = meta["EP"], meta["NSTG"]
    win_tiles, nstg = meta["win_tiles"], meta["nstg"]
    tile_block = meta["tile_block"]
    tile_first, tile_last = meta["tile_first"], meta["tile_last"]
    firstw = meta["firstw"]
    AGG = IN + H   # 132 cols: numer | denom

    nc = bacc.Bacc("TRN2", target_bir_lowering=False, debug=False,
                   enable_asserts=False, num_devices=8)
    tblT = nc.dram_tensor("tblT", [128, NT], F16, kind="ExternalInput")
    fq16T = nc.dram_tensor("fq16T", [128, NQP], F16, kind="ExternalInput")
    fq32 = nc.dram_tensor("fq32", [128, NB * 128], F32, kind="ExternalInput")
    Wz16 = nc.dram_tensor("Wz16", [IN, IN], F16, kind="ExternalInput")
    War16 = nc.dram_tensor("War16", [IN, IN], F16, kind="ExternalInput")
    bias_b = nc.dram_tensor("bias_b", [128, IN], F32, kind="ExternalInput")
    inval_b = nc.dram_tensor("inval_b", [128, IN], F32, kind="ExternalInput")
    iota_b16 = nc.dram_tensor("iota_b16", [128, 128], F16, kind="ExternalInput")
    iota_colf = nc.dram_tensor("iota_colf", [128, 1], F32, kind="ExternalInput")
    idxp = nc.dram_tensor("idxp", [128, EP // 16], I16, kind="ExternalInput")
    dcol = nc.dram_tensor("dcol", [NSTG, 128, ST], F32, kind="ExternalInput")
    drow = nc.dram_tensor("drow", [NSTG, 128 * ST], F16, kind="ExternalInput")
    out = nc.dram_tensor("out", [NQP, IN], F32, kind="ExternalOutput")
    ztabs = [nc.dram_tensor(f"ztab{w}", [WS, IN], F16) for w in range(NW)]

    with TileContext(nc) as tc:
        nc.gpsimd.load_library(library_config.mlp)
        with tc.tile_pool(name="consts", bufs=1) as cpool, \
             tc.tile_pool(name="acc", bufs=1) as apool:
            # resident tiles
            acc = apool.tile([128, NB * AGG], F32, tag="acc")
            er16 = cpool.tile([128, NB * H], F16, tag="er16")
            wz = cpool.tile([IN, IN], F16, tag="wz")
            war = cpool.tile([IN, IN], F16, tag="war")
            iob = cpool.tile([128, 128], F16, tag="iob")
            ioc = cpool.tile([128, 1], F32, tag="ioc")
            alpha = cpool.tile([128, 1], F32, tag="alpha")
            invt = cpool.tile([128, 128], F32, tag="invt")
            biast = cpool.tile([128, 128], F32, tag="biast")
            nc.sync.dma_start(invt[:, :], inval_b[:, :])
            nc.sync.dma_start(biast[:, :], bias_b[:, :])
            nc.sync.dma_start(wz[:, :], Wz16[:, :])
            nc.sync.dma_start(war[:, :], War16[:, :])
            nc.sync.dma_start(iob[:, :], iota_b16[:, :])
            nc.sync.dma_start(ioc[:, :], iota_colf[:, :])
            nc.vector.memset(alpha[:, :], cfg["SLOPE"])

            # ---------------- P1: zal table + er ----------------------
            CH = min(16, NT // 128)   # blocks (128 rows each) per chunk
            assert NT % (128 * CH) == 0 and CH % 4 == 0
            nchunk = NT // (128 * CH)
            with tc.tile_pool(name="p1", bufs=2) as p1, \
                 tc.tile_pool(name="p1ps", bufs=2, space="PSUM") as p1ps:
                for c in range(nchunk):
                    fin = p1.tile([128, 128 * CH], F16, tag="fin")
                    nc.sync.dma_start(fin[:, :], tblT[:, 128 * CH * c:128 * CH * (c + 1)])
                    cp = p1.tile([128, CH, IN], F16, tag="cp")
                    for g in range(CH // 4):
                        ps = p1ps.tile([128, 512], F32, tag="ps")
                        for j in range(4):
                            nc.tensor.matmul(ps[:, 128 * j:128 * (j + 1)],
                                             fin[:, 512 * g + 128 * j:512 * g + 128 * (j + 1)],
                                             wz[:, :], start=True, stop=True)
                        nc.vector.tensor_copy(
                            cp[:, 4 * g:4 * (g + 1), :].rearrange("p a b -> p (a b)"),
                            ps[:, :])
                    blk0 = CH * c
                    WB = WS // 128
                    done = 0
                    while done < CH:
                        w0 = (blk0 + done) // WB
                        wb = (blk0 + done) % WB
                        take = min(CH - done, WB - wb)
                        nc.sync.dma_start(
                            ztabs[w0].ap().rearrange("(c p) f -> p c f", p=128)[:, wb:wb + take, :],
                            cp[:, done:done + take, :])
                        done += take
                # er: feat_tgt quarter @ War, head-summed
                ERCH = 4  # blocks per chunk
                for c in range(NB // ERCH + (1 if NB % ERCH else 0)):
                    blo = ERCH * c
                    bn = min(ERCH, NB - blo)
                    feq = p1.tile([128, 512], F16, tag="feq")
                    nc.sync.dma_start(feq[:, :128 * bn],
                                      fq16T[:, 128 * blo:128 * (blo + bn)])
                    ps = p1ps.tile([128, 512], F32, tag="ps2")
                    for j in range(bn):
                        nc.tensor.matmul(ps[:, 128 * j:128 * (j + 1)],
                                         feq[:, 128 * j:128 * (j + 1)],
                                         war[:, :], start=True, stop=True)
                    er32 = p1.tile([128, ERCH * H], F32, tag="er32")
                    nc.vector.tensor_reduce(
                        er32[:, :bn * H].rearrange("p (b h) -> p b h", h=H),
                        ps[:, :128 * bn].rearrange("p (b h d) -> p b h d", h=H, d=D),
                        axis=mybir.AxisListType.X, op=ALU.add)
                    nc.vector.tensor_copy(er16[:, H * blo:H * (blo + bn)],
                                          er32[:, :bn * H])

            # ---------------- P2: edge pipeline ------------------------
            with tc.tile_pool(name="p2", bufs=2) as p2, \
                 tc.tile_pool(name="sp", bufs=4) as sp, \
                 tc.tile_pool(name="p2ps", bufs=2, space="PSUM") as p2ps, \
                 tc.tile_pool(name="erps_p", bufs=2, space="PSUM") as erpsp:
                skip_p2 = "2" not in PH
                a = 0      # edge offset
                sg = 0     # global stage
                for w in range(NW):
                    wt = win_tiles[w]
                    ti = 0   # tile index within window
                    slotps = None
                    for s0 in range(0, wt, ST):
                        if skip_p2:
                            break
                        n_t = min(ST, wt - s0)
                        n_e = 128 * n_t
                        ixs = p2.tile([128, ST * 8], I16, tag="ixs")
                        nc.sync.dma_start(ixs[:, :n_e // 16],
                                          idxp[:, a // 16:(a + n_e) // 16])
                        stg = p2.tile([128, ST, IN], F16, tag="stg")
                        if "nogather" in SUB:
                            nc.vector.memset(stg[:, :n_t, :], 0.5)
                        else:
                            nc.gpsimd.dma_gather(stg[:, :n_t, :],
                                                 ztabs[w].ap(),
                                                 ixs[:, :n_e // 16], n_e, n_e, IN,
                                                 single_packet=os.environ.get("KSP", "0") == "1")
                        drc = p2.tile([128, ST], F32, tag="drc")
                        nc.sync.dma_start(drc[:, :n_t], dcol.ap()[sg, :, :n_t])
                        drb = p2.tile([128, 128 * ST], F16, tag="drb")
                        if "nobcast" in SUB:
                            nc.vector.memset(drb[:, :n_e], 1.0)
                        else:
                            nc.sync.dma_start(drb[0:1, :n_e], drow.ap()[sg:sg + 1, :n_e])
                            nc.gpsimd.partition_broadcast(drb[:, :n_e], drb[0:1, :n_e])
                        # el
                        elt = p2.tile([128, ST * H], F32, tag="elt")
                        nc.vector.tensor_reduce(
                            elt[:, :n_t * H].rearrange("p (t h) -> p t h", h=H),
                            stg[:, :n_t, :].rearrange("p t (d h) -> p t h d", h=H),
                            axis=mybir.AxisListType.X, op=ALU.add)
                        # er per edge via one-hot matmuls
                        esum = p2.tile([128, ST * H], F32, tag="esum")
                        if "noer" in SUB:
                            nc.vector.tensor_copy(esum[:, :n_t * H], elt[:, :n_t * H])
                        else:
                            erps = erpsp.tile([128, ST * H], F32, tag="erps")
                            sbc = p2.tile([128, ST * 128], F16, tag="sbc")
                            nc.vector.tensor_scalar(sbc[:, :n_e], drb[:, :n_e],
                                                    ioc[:, :], None, op0=ALU.is_equal)
                            for t in range(n_t):
                                blk = tile_block[w][ti + t]
                                nc.tensor.matmul(erps[:, H * t:H * (t + 1)],
                                                 sbc[:, 128 * t:128 * (t + 1)],
                                                 er16[:, H * blk:H * (blk + 1)],
                                                 start=True, stop=True)
                            nc.vector.tensor_tensor(esum[:, :n_t * H], elt[:, :n_t * H],
                                                    erps[:, :n_t * H], op=ALU.add)
                        esum2 = p2.tile([128, ST * H], F32, tag="esum2")
                        nc.vector.scalar_tensor_tensor(
                            esum2[:, :n_t * H], esum[:, :n_t * H], cfg["SLOPE"],
                            esum[:, :n_t * H], op0=ALU.mult, op1=ALU.max)
                        rhs = p2.tile([128, ST, AGG], F16, tag="rhs")
                        if "noexp" in SUB:
                            nc.vector.memset(rhs[:, :n_t, IN:], 1.0)
                        else:
                            nc.scalar.activation(
                                rhs[:, :n_t, IN:],
                                esum2[:, :n_t * H].rearrange("p (t h) -> p t h", h=H),
                                AF.Exp)
                        for t in range(n_t):
                            # exz
                            exb = rhs[:, t:t + 1, IN:].broadcast_to((128, D, H))
                            nc.vector.tensor_tensor(
                                rhs[:, t, :IN].rearrange("p (d h) -> p d h", h=H),
                                stg[:, t, :].rearrange("p (d h) -> p d h", h=H),
                                exb, op=ALU.mult)
                            sred = sp.tile([128, 128], F16, tag="sred")
                            nc.vector.tensor_scalar(sred[:, :], iob[:, :],
                                                    drc[:, t:t + 1], None,
                                                    op0=ALU.is_equal)
                            blk = tile_block[w][ti + t]
                            if tile_first[w][ti + t]:
                                slotps = p2ps.tile([128, AGG], F32, tag="slot")
                            nc.tensor.matmul(slotps[:, :], sred[:, :], rhs[:, t, :],
                                             start=tile_first[w][ti + t],
                                             stop=tile_last[w][ti + t])
                            if tile_last[w][ti + t]:
                                dst = acc[:, AGG * blk:AGG * (blk + 1)]
                                if firstw[blk] == w:
                                    nc.vector.tensor_copy(dst, slotps[:, :])
                                else:
                                    nc.vector.tensor_tensor(dst, slotps[:, :], dst,
                                                            op=ALU.add)
                        ti += n_t
                        a += n_e
                        sg += 1

            # ---------------- P3: normalize + residual + relu ----------
            with tc.tile_pool(name="p3", bufs=2) as p3:
                # blocks never touched: zero them
                for b in range(NB):
                    if firstw[b] < 0 or skip_p2:
                        nc.vector.memset(acc[:, AGG * b:AGG * (b + 1)], 0.0)
                den = p3.tile([128, NB * H], F32, tag="den")
                accv = acc[:, :].rearrange("p (b g) -> p b g", g=AGG)
                nc.vector.tensor_scalar(
                    den[:, :].rearrange("p (b h) -> p b h", h=H),
                    accv[:, :, IN:], 1e-9, None, op0=ALU.max)
                rden = p3.tile([128, NB * H], F32, tag="rden")
                nc.vector.reciprocal(rden[:, :], den[:, :])
                P3C = 14   # blocks per chunk
                for c in range(NB // P3C + (1 if NB % P3C else 0)):
                    blo = P3C * c
                    bn = min(P3C, NB - blo)
                    fch = p3.tile([128, P3C * 128], F32, tag="fch")
                    nc.sync.dma_start(fch[:, :bn * 128],
                                      fq32[:, 128 * blo:128 * (blo + bn)])
                    t1 = p3.tile([128, P3C * 128], F32, tag="t1")
                    # unpermute numer (d*H+h -> h*D+d) and scale by 1/denom
                    nperm = accv[:, blo:blo + bn, :IN].rearrange(
                        "p b (d h) -> p b h d", h=H)
                    rexp = rden[:, H * blo:H * (blo + bn)].rearrange(
                        "p (b h) -> p b h", h=H).unsqueeze(3).broadcast_to(
                        (128, bn, H, D))
                    nc.vector.tensor_tensor(
                        t1[:, :bn * 128].rearrange("p (b h d) -> p b h d", h=H, d=D),
                        nperm, rexp, op=ALU.mult)
                    t2 = p3.tile([128, P3C * 128], F32, tag="t2")
                    ib = invt[:, :].unsqueeze(1).broadcast_to((128, bn, IN))
                    nc.vector.tensor_tensor(
                        t2[:, :bn * 128].rearrange("p (b f) -> p b f", f=IN),
                        t1[:, :bn * 128].rearrange("p (b f) -> p b f", f=IN),
                        ib, op=ALU.mult)
                    nc.vector.tensor_tensor(t1[:, :bn * 128], t2[:, :bn * 128],
                                            fch[:, :bn * 128], op=ALU.add)
                    bb = biast[:, :].unsqueeze(1).broadcast_to((128, bn, IN))
                    nc.vector.tensor_tensor(
                        t2[:, :bn * 128].rearrange("p (b f) -> p b f", f=IN),
                        t1[:, :bn * 128].rearrange("p (b f) -> p b f", f=IN),
                        bb, op=ALU.add)
                    nc.vector.tensor_scalar(t1[:, :bn * 128], t2[:, :bn * 128],
                                            0.0, None, op0=ALU.max)
                    nc.sync.dma_start(
                        out.ap().rearrange("(b p) f -> p b f", p=128)[:, blo:blo + bn, :],
                        t1[:, :bn * 128].rearrange("p (b f) -> p b f", f=IN))
    nc.compile()
    return nc


LAST_EXEC_NS = None


def kernel(**inputs):
    global LAST_EXEC_NS
    cfg = CFG
    in_maps, meta = host_prep(inputs, cfg)
    nc = build_program(meta)
    import time as _time
    trace = os.environ.get("KTRACE", "0") == "1"
    _t0 = _time.time()
    try:
        res = run_bass_kernel_spmd(nc, in_maps, core_ids=list(range(8)), trace=trace)
    except ModuleNotFoundError:
        res = run_bass_kernel_spmd(nc, in_maps, core_ids=list(range(8)))
    _wall = _time.time() - _t0
    LAST_EXEC_NS = res.exec_time_ns or int(_wall * 1e9)
    outs = res.results
    NQ = cfg["N"] // 4
    full = np.empty((cfg["N"], cfg["IN"]), np.float32)
    for q in range(4):
        full[NQ * q:NQ * (q + 1)] = (outs[q]["out"][:NQ]
                                     + outs[4 + q]["out"][:NQ])
    return full
